# revision 16
# baseline (speedup 1.0000x reference)
"""Trainium2 Bass kernel: topk_masking Adam-loop (nn_AAALinear_6347961663813).

Data-parallel over 8 NeuronCores: each core takes 512 rows of x, computes
logits = x @ W + b (TensorE, float32r full-rate), then runs 100 fully
unrolled Adam iterations of the analytic gradient of
    loss = |margin(lg) - target|.mean() + 5*|softmax(lg)[j*] - p_ori|.mean()
entirely in SBUF. Layout: row r of the shard lives at partition r%128,
row-group g = r//128 (tiles are [128, 4, 1000]).

Analytic gradient per row (validated vs jax.grad):
    g = sd*(e_a - e_b) + cc*(e_j* - prob),  sd = sign+(margin-target)/B,
    cc = 5*sign+(pstar-p_ori)*pstar/B,      sign+(x) = x>=0 ? 1 : -1
where a/b are the current top-2 positions (realized as equality masks
against the top-2 values from the Max8 instruction) and j* is the fixed
original argmax. sqrt/reciprocal in the Adam denominator are computed as
exp/ln chains so the ScalarE table set never switches.
"""
import sys

for _p in ("/opt/trn_rl_repo", "/root/.axon_site/_ro/trn_rl_repo"):
    if _p not in sys.path:
        sys.path.insert(0, _p)

import numpy as np

import concourse.bass as bass
import concourse.mybir as mybir
import concourse.tile as tile
from concourse import bacc
from concourse.bass import MemorySpace
from concourse.bass_utils import run_bass_kernel_spmd

# All activation funcs we use (Exp, Ln, Square, Copy, Identity, Sign) live in
# the natural_log_exp_and_others table set. The default chooser alternates
# between exp_and_others and natural_log_exp_and_others, inserting ~6 table
# loads (~2.7us each) per loop iteration. Empty every other set's function
# list (keeping dict order, which defines act_func_set_id) so the chooser can
# only pick the combined set; the load then hoists out of the loop.
_orig_get_activation_tables = bacc.get_activation_tables


def _pinned_get_activation_tables(module_arch):
    tables = _orig_get_activation_tables(module_arch)
    pinned = {}
    for name, fns in tables.items():
        pinned[name] = fns if name == "natural_log_exp_and_others" else set()
    return pinned


bacc.get_activation_tables = _pinned_get_activation_tables

B, D, C = 4096, 2048, 1000
NCORES = 8
R = B // NCORES  # 512 rows per core
P = 128
G = R // P  # 4 row groups
KT = D // P  # 16 k-tiles
NUM_ITER = 100
LR, BETA1, BETA2, EPS, CAL_W = 0.1, 0.9, 0.999, 1e-8, 5.0

f32 = mybir.dt.float32
f32r = mybir.dt.float32r
AF = mybir.ActivationFunctionType
OP = mybir.AluOpType

_cache = {}


def build(num_iter=NUM_ITER):
    nc = bacc.Bacc(None, target_bir_lowering=False, debug=False)

    xT = nc.declare_dram_parameter("xT", [D, R], f32, isOutput=False)
    Wp = nc.declare_dram_parameter("W", [D, C], f32, isOutput=False)
    bp = nc.declare_dram_parameter("b", [1, C], f32, isOutput=False)
    out = nc.declare_dram_parameter("out", [R, C], f32, isOutput=True)

    K1 = (1.0 - BETA1) / B          # folds (1-beta1) of the m update into g
    K2 = CAL_W * (1.0 - BETA1) / B
    K3 = float(np.sqrt(1.0 - BETA2) / (1.0 - BETA1))  # Square(K3*gs) = (1-b2)*g^2

    with tile.TileContext(nc) as tc:
        with tc.tile_pool(name="persist", bufs=1) as pp:
            lg = pp.tile([P, G, C], f32)
            m = pp.tile([P, G, C], f32)
            v = pp.tile([P, G, C], f32)
            maskf = pp.tile([P, G, C], mybir.dt.bfloat16)
            expv = pp.tile([P, G, C], mybir.dt.bfloat16)
            v8 = pp.tile([P, G, 8], f32)
            # row scalars [P, G]
            negmax = pp.tile([P, G], f32)
            sumexp = pp.tile([P, G], f32)
            p_ori = pp.tile([P, G], f32)
            target = pp.tile([P, G], f32)
            pn = pp.tile([P, G], f32)
            rsum = pp.tile([P, G], f32)
            pstar = pp.tile([P, G], f32)
            dcal = pp.tile([P, G], f32)
            marg = pp.tile([P, G], f32)
            dmt = pp.tile([P, G], f32)
            sd = pp.tile([P, G], f32)
            sdn = pp.tile([P, G], f32)
            cc = pp.tile([P, G], f32)
            negr = pp.tile([P, G], f32)
            t0 = pp.tile([P, G], f32)
            # per-iteration bias constants ln(LR / (1 - beta1^t))
            lnk5_sb = pp.tile([P, num_iter], f32)
            for t in range(1, num_iter + 1):
                val = float(np.log(LR / (1.0 - BETA1 ** t)))
                nc.vector.memset(lnk5_sb[:, t - 1:t], val)
            eps_sb = pp.tile([P, 1], f32)
            nc.vector.memset(eps_sb[:], EPS)
            ident_f = pp.tile([P, P], f32)
            ident = pp.tile([P, P], f32r)
            from concourse.masks import make_identity
            make_identity(nc, ident_f[:])
            nc.vector.tensor_copy(ident[:], ident_f[:])

            # ---------- matmul: logits = x @ W + b ----------
            with (
                tc.tile_pool(name="mm", bufs=1) as mmp,
                tc.tile_pool(name="psum", bufs=4, space=MemorySpace.PSUM) as psp,
            ):
                w_sb = mmp.tile([P, KT, C], f32)
                nc.sync.dma_start(w_sb[:], Wp[:].rearrange("(k p) c -> p k c", p=P))
                xt_sb = mmp.tile([P, KT, R], f32)
                nc.sync.dma_start(xt_sb[:], xT[:].rearrange("(k p) r -> p k r", p=P))
                b_sb = mmp.tile([1, C], f32)
                nc.sync.dma_start(b_sb[:], bp[:])
                ones = mmp.tile([1, P], f32)
                nc.vector.memset(ones[:], 1.0)

                for g in range(G):
                    for n0, nw in ((0, 500), (500, 500)):
                        ps = psp.tile([P, 512], f32, tag="ps")
                        for k in range(KT):
                            nc.tensor.matmul(
                                ps[:, :nw],
                                xt_sb[:, k, g * P:(g + 1) * P],
                                w_sb[:, k, n0:n0 + nw],
                                start=(k == 0),
                                stop=False,
                            )
                        nc.tensor.matmul(
                            ps[:, :nw],
                            ones[:],
                            b_sb[:, n0:n0 + nw],
                            start=False,
                            stop=True,
                        )
                        nc.scalar.copy(lg[:, g, n0:n0 + nw], ps[:, :nw])

            # ---------- setup: maskf, target, p_ori ----------
            for g in range(G):
                nc.vector.max(out=v8[:, g, :], in_=lg[:, g, :])
            nc.vector.tensor_scalar_mul(negmax[:], v8[:, :, 0], -1.0)
            for g in range(G):
                nc.vector.tensor_scalar(
                    out=maskf[:, g, :], in0=lg[:, g, :],
                    scalar1=v8[:, g, 0:1], scalar2=None, op0=OP.is_equal,
                )
            # target = 2*attractor - margin; attractor = 4*round_half_even(q) - 2,
            # q = margin/4 + 0.5. round_half_even via the fp32 (x+2^23)-2^23 trick.
            MAGIC = float(2 ** 23)
            nc.vector.tensor_sub(marg[:], v8[:, :, 0], v8[:, :, 1])
            nc.vector.tensor_scalar(out=t0[:], in0=marg[:], scalar1=0.25,
                                    scalar2=0.5, op0=OP.mult, op1=OP.add)
            nc.vector.tensor_scalar(out=t0[:], in0=t0[:], scalar1=MAGIC,
                                    scalar2=-MAGIC, op0=OP.add, op1=OP.add)
            # attractor = 4*rhe - 2; 2*attractor = 8*rhe - 4
            nc.vector.tensor_scalar(out=t0[:], in0=t0[:], scalar1=8.0,
                                    scalar2=-4.0, op0=OP.mult, op1=OP.add)
            nc.vector.tensor_sub(target[:], t0[:], marg[:])
            # p_ori = 1/sum(exp(lg - max))
            for g in range(G):
                nc.scalar.activation(expv[:, g, :], lg[:, g, :], AF.Exp,
                                     bias=negmax[:, g:g + 1], scale=1.0,
                                     accum_out=sumexp[:, g:g + 1])
            nc.vector.reciprocal(p_ori[:], sumexp[:])
            nc.vector.memset(m[:], 0.0)
            nc.vector.memset(v[:], 0.0)

            # ---------- Adam loop ----------
            with (
                tc.tile_pool(name="loop", bufs=1) as lp,
                tc.tile_pool(name="lpsum", bufs=6, space=MemorySpace.PSUM) as lps,
            ):
                scr = lp.tile([P, G, C], mybir.dt.bfloat16)
                gs = lp.tile([P, G, C], f32)
                sq = lp.tile([P, G, C], f32)
                tA = lp.tile([P, G, C], f32r)
                tB = lp.tile([P, G, C], f32r)
                tU = lp.tile([P, G, C], f32r)
                tW = lp.tile([P, G, C], f32r)
                NT = ((0, 500), (500, 500))

                for t in range(1, num_iter + 1):
                    bc2 = 1.0 - BETA2 ** t
                    k4 = 1.0 / bc2

                    # stage-major emission: all groups per stage, so the
                    # scheduler's program-order priorities align with
                    # cross-group pipelining
                    for g in range(G):
                        gc = slice(g, g + 1)
                        nc.vector.max(out=v8[:, g, :], in_=lg[:, g, :])
                        nc.vector.tensor_scalar_mul(negmax[:, gc], v8[:, g, 0:1], -1.0)
                        nc.scalar.activation(expv[:, g, :], lg[:, g, :], AF.Exp,
                                             bias=negmax[:, gc], scale=1.0,
                                             accum_out=sumexp[:, gc])
                        # NB: tensor_tensor_reduce faults TRN2; this
                        # scalar_tensor_tensor+accum form is equivalent.
                        nc.vector.scalar_tensor_tensor(
                            out=scr[:, g, :], in0=expv[:, g, :], scalar=1.0,
                            in1=maskf[:, g, :], op0=OP.mult, op1=OP.mult,
                            accum_out=pn[:, gc],
                        )
                    for g in range(G):
                        gc = slice(g, g + 1)
                        nc.vector.reciprocal(rsum[:, gc], sumexp[:, gc])
                        nc.vector.tensor_mul(pstar[:, gc], pn[:, gc], rsum[:, gc])
                        nc.vector.tensor_sub(dcal[:, gc], pstar[:, gc], p_ori[:, gc])
                        nc.vector.tensor_sub(marg[:, gc], v8[:, g, 0:1], v8[:, g, 1:2])
                        nc.vector.tensor_sub(dmt[:, gc], marg[:, gc], target[:, gc])
                        nc.vector.tensor_scalar(out=sd[:, gc], in0=dmt[:, gc],
                                                scalar1=0.0, scalar2=2.0 * K1,
                                                op0=OP.is_ge, op1=OP.mult)
                        nc.vector.tensor_scalar_add(sd[:, gc], sd[:, gc], -K1)
                        nc.vector.tensor_scalar_mul(sdn[:, gc], sd[:, gc], -1.0)
                        nc.vector.tensor_scalar(out=t0[:, gc], in0=dcal[:, gc],
                                                scalar1=0.0, scalar2=2.0 * K2,
                                                op0=OP.is_ge, op1=OP.mult)
                        nc.vector.tensor_scalar_add(t0[:, gc], t0[:, gc], -K2)
                        nc.vector.tensor_mul(cc[:, gc], t0[:, gc], pstar[:, gc])
                        nc.vector.tensor_mul(t0[:, gc], cc[:, gc], rsum[:, gc])
                        nc.vector.tensor_scalar_mul(negr[:, gc], t0[:, gc], -1.0)
                    for g in range(G):
                        gc = slice(g, g + 1)
                        nc.vector.tensor_scalar(
                            out=tA[:, g, :], in0=lg[:, g, :],
                            scalar1=v8[:, g, 0:1], scalar2=sd[:, gc],
                            op0=OP.is_equal, op1=OP.mult,
                        )
                        nc.vector.tensor_scalar(
                            out=tB[:, g, :], in0=lg[:, g, :],
                            scalar1=v8[:, g, 1:2], scalar2=sdn[:, gc],
                            op0=OP.is_equal, op1=OP.mult,
                        )
                        nc.scalar.activation(tU[:, g, :], expv[:, g, :],
                                             AF.Copy, bias=0.0, scale=negr[:, gc])
                        nc.scalar.activation(tW[:, g, :], maskf[:, g, :],
                                             AF.Copy, bias=0.0, scale=cc[:, gc])
                    for g in range(G):
                        gc = slice(g, g + 1)
                        for n0, nw in NT:
                            gs_ps = lps.tile([P, 512], f32, tag="gs")
                            ns = slice(n0, n0 + nw)
                            nc.tensor.matmul(gs_ps[:, :nw], ident[:], tA[:, g, ns],
                                             start=True, stop=False)
                            nc.tensor.matmul(gs_ps[:, :nw], ident[:], tB[:, g, ns],
                                             start=False, stop=False)
                            nc.tensor.matmul(gs_ps[:, :nw], ident[:], tU[:, g, ns],
                                             start=False, stop=False)
                            nc.tensor.matmul(gs_ps[:, :nw], ident[:], tW[:, g, ns],
                                             start=False, stop=True)
                            nc.vector.scalar_tensor_tensor(
                                out=m[:, g, ns], in0=m[:, g, ns], scalar=BETA1,
                                in1=gs_ps[:, :nw], op0=OP.mult, op1=OP.add,
                            )
                            nc.scalar.activation(sq[:, g, ns], gs_ps[:, :nw],
                                                 AF.Square, bias=0.0, scale=K3)
                        nc.vector.scalar_tensor_tensor(
                            out=v[:, g, :], in0=v[:, g, :], scalar=BETA2,
                            in1=sq[:, g, :], op0=OP.mult, op1=OP.add,
                        )
                    for g in range(G):
                        gc = slice(g, g + 1)
                        nc.scalar.activation(sq[:, g, :], v[:, g, :], AF.Ln,
                                             bias=0.0, scale=k4)
                        nc.scalar.activation(gs[:, g, :], sq[:, g, :], AF.Exp,
                                             bias=0.0, scale=0.5)
                        nc.scalar.activation(sq[:, g, :], gs[:, g, :], AF.Ln,
                                             bias=eps_sb[:], scale=1.0)
                        nc.scalar.activation(sq[:, g, :], sq[:, g, :], AF.Exp,
                                             bias=lnk5_sb[:, t - 1:t], scale=-1.0)
                        nc.vector.tensor_mul(gs[:, g, :], m[:, g, :], sq[:, g, :])
                        nc.vector.tensor_sub(lg[:, g, :], lg[:, g, :], gs[:, g, :])

            nc.sync.dma_start(out[:].rearrange("(g p) c -> p g c", p=P), lg[:])

    nc.compile()
    return nc


def _get_nc(num_iter=NUM_ITER):
    if num_iter not in _cache:
        _cache[num_iter] = build(num_iter)
    return _cache[num_iter]


def kernel(x, W, b, num_iter=NUM_ITER, trace=False):
    x = np.ascontiguousarray(x, dtype=np.float32)
    W = np.ascontiguousarray(W, dtype=np.float32)
    b = np.ascontiguousarray(b, dtype=np.float32).reshape(1, C)
    nc = _get_nc(num_iter)
    in_maps = []
    for i in range(NCORES):
        xt = np.ascontiguousarray(x[i * R:(i + 1) * R].T)
        in_maps.append({"xT": xt, "W": W, "b": b})
    res = run_bass_kernel_spmd(nc, in_maps, core_ids=list(range(NCORES)),
                               trace=trace)
    out = np.concatenate([res.results[i]["out"] for i in range(NCORES)], axis=0)
    if trace:
        kernel.last_exec_time_ns = res.exec_time_ns
        kernel.last_result = res
    return out


# revision 17
# speedup vs baseline: 1.1743x; 1.1743x over previous
"""Trainium2 Bass kernel: topk_masking Adam-loop (nn_AAALinear_6347961663813).

Data-parallel over 8 NeuronCores: each core takes 512 rows of x, computes
logits = x @ W + b (TensorE, float32r full-rate), then runs 100 fully
unrolled Adam iterations of the analytic gradient of
    loss = |margin(lg) - target|.mean() + 5*|softmax(lg)[j*] - p_ori|.mean()
entirely in SBUF. Layout: row r of the shard lives at partition r%128,
row-group g = r//128 (tiles are [128, 4, 1000]).

Analytic gradient per row (validated vs jax.grad):
    g = sd*(e_a - e_b) + cc*(e_j* - prob),  sd = sign+(margin-target)/B,
    cc = 5*sign+(pstar-p_ori)*pstar/B,      sign+(x) = x>=0 ? 1 : -1
where a/b are the current top-2 positions (realized as equality masks
against the top-2 values from the Max8 instruction) and j* is the fixed
original argmax. sqrt/reciprocal in the Adam denominator are computed as
exp/ln chains so the ScalarE table set never switches.
"""
import sys

for _p in ("/opt/trn_rl_repo", "/root/.axon_site/_ro/trn_rl_repo"):
    if _p not in sys.path:
        sys.path.insert(0, _p)

import numpy as np

import concourse.bass as bass
import concourse.mybir as mybir
import concourse.tile as tile
from concourse import bacc
from concourse.bass import MemorySpace
from concourse.bass_utils import run_bass_kernel_spmd

# All activation funcs we use (Exp, Ln, Square, Copy, Identity, Sign) live in
# the natural_log_exp_and_others table set. The default chooser alternates
# between exp_and_others and natural_log_exp_and_others, inserting ~6 table
# loads (~2.7us each) per loop iteration. Empty every other set's function
# list (keeping dict order, which defines act_func_set_id) so the chooser can
# only pick the combined set; the load then hoists out of the loop.
_orig_get_activation_tables = bacc.get_activation_tables


def _pinned_get_activation_tables(module_arch):
    tables = _orig_get_activation_tables(module_arch)
    pinned = {}
    for name, fns in tables.items():
        pinned[name] = fns if name == "natural_log_exp_and_others" else set()
    return pinned


bacc.get_activation_tables = _pinned_get_activation_tables

B, D, C = 4096, 2048, 1000
NCORES = 8
R = B // NCORES  # 512 rows per core
P = 128
G = R // P  # 4 row groups
KT = D // P  # 16 k-tiles
NUM_ITER = 100
LR, BETA1, BETA2, EPS, CAL_W = 0.1, 0.9, 0.999, 1e-8, 5.0

f32 = mybir.dt.float32
f32r = mybir.dt.float32r
AF = mybir.ActivationFunctionType
OP = mybir.AluOpType

_cache = {}


def build(num_iter=NUM_ITER):
    nc = bacc.Bacc(None, target_bir_lowering=False, debug=False)

    xT = nc.declare_dram_parameter("xT", [D, R], f32, isOutput=False)
    Wp = nc.declare_dram_parameter("W", [D, C], f32, isOutput=False)
    bp = nc.declare_dram_parameter("b", [1, C], f32, isOutput=False)
    out = nc.declare_dram_parameter("out", [R, C], f32, isOutput=True)

    K1 = (1.0 - BETA1) / B          # folds (1-beta1) of the m update into g
    K2 = CAL_W * (1.0 - BETA1) / B
    K3 = float(np.sqrt(1.0 - BETA2) / (1.0 - BETA1))  # Square(K3*gs) = (1-b2)*g^2

    with tile.TileContext(nc) as tc:
        with tc.tile_pool(name="persist", bufs=1) as pp:
            lg = pp.tile([P, G, C], f32)
            m = pp.tile([P, G, C], f32)
            v = pp.tile([P, G, C], f32)
            maskf = pp.tile([P, G, C], f32)
            expv = pp.tile([P, G, C], f32)
            v8 = pp.tile([P, G, 8], f32)
            # row scalars [P, G]
            negmax = pp.tile([P, G], f32)
            sumexp = pp.tile([P, G], f32)
            p_ori = pp.tile([P, G], f32)
            target = pp.tile([P, G], f32)
            pn = pp.tile([P, G], f32)
            rsum = pp.tile([P, G], f32)
            pstar = pp.tile([P, G], f32)
            dcal = pp.tile([P, G], f32)
            marg = pp.tile([P, G], f32)
            dmt = pp.tile([P, G], f32)
            sd = pp.tile([P, G], f32)
            sdn = pp.tile([P, G], f32)
            cc = pp.tile([P, G], f32)
            negr = pp.tile([P, G], f32)
            t0 = pp.tile([P, G], f32)
            # per-iteration bias constants ln(LR / (1 - beta1^t))
            lnk5_sb = pp.tile([P, num_iter], f32)
            for t in range(1, num_iter + 1):
                val = float(np.log(LR / (1.0 - BETA1 ** t)))
                nc.vector.memset(lnk5_sb[:, t - 1:t], val)
            eps_sb = pp.tile([P, 1], f32)
            nc.vector.memset(eps_sb[:], EPS)
            ident_f = pp.tile([P, P], f32)
            ident = pp.tile([P, P], f32r)
            from concourse.masks import make_identity
            make_identity(nc, ident_f[:])
            nc.vector.tensor_copy(ident[:], ident_f[:])

            # ---------- matmul: logits = x @ W + b ----------
            with (
                tc.tile_pool(name="mm", bufs=1) as mmp,
                tc.tile_pool(name="psum", bufs=4, space=MemorySpace.PSUM) as psp,
            ):
                w_sb = mmp.tile([P, KT, C], f32)
                xt_sb = mmp.tile([P, KT, R], f32)
                Wr = Wp[:].rearrange("(k p) c -> p k c", p=P)
                Xr = xT[:].rearrange("(k p) r -> p k r", p=P)
                for k in range(KT):
                    nc.sync.dma_start(xt_sb[:, k, :], Xr[:, k, :])
                    nc.sync.dma_start(w_sb[:, k, :], Wr[:, k, :])
                b_sb = mmp.tile([1, C], f32)
                nc.sync.dma_start(b_sb[:], bp[:])
                ones = mmp.tile([1, P], f32)
                nc.vector.memset(ones[:], 1.0)

                for g in range(G):
                    for n0, nw in ((0, 500), (500, 500)):
                        ps = psp.tile([P, 512], f32, tag="ps")
                        for k in range(KT):
                            nc.tensor.matmul(
                                ps[:, :nw],
                                xt_sb[:, k, g * P:(g + 1) * P],
                                w_sb[:, k, n0:n0 + nw],
                                start=(k == 0),
                                stop=False,
                            )
                        nc.tensor.matmul(
                            ps[:, :nw],
                            ones[:],
                            b_sb[:, n0:n0 + nw],
                            start=False,
                            stop=True,
                        )
                        nc.scalar.copy(lg[:, g, n0:n0 + nw], ps[:, :nw])

            # ---------- setup: maskf, target, p_ori ----------
            for g in range(G):
                nc.vector.max(out=v8[:, g, :], in_=lg[:, g, :])
            nc.vector.tensor_scalar_mul(negmax[:], v8[:, :, 0], -1.0)
            for g in range(G):
                nc.vector.tensor_scalar(
                    out=maskf[:, g, :], in0=lg[:, g, :],
                    scalar1=v8[:, g, 0:1], scalar2=None, op0=OP.is_equal,
                )
            # target = 2*attractor - margin; attractor = 4*round_half_even(q) - 2,
            # q = margin/4 + 0.5. round_half_even via the fp32 (x+2^23)-2^23 trick.
            MAGIC = float(2 ** 23)
            nc.vector.tensor_sub(marg[:], v8[:, :, 0], v8[:, :, 1])
            nc.vector.tensor_scalar(out=t0[:], in0=marg[:], scalar1=0.25,
                                    scalar2=0.5, op0=OP.mult, op1=OP.add)
            nc.vector.tensor_scalar(out=t0[:], in0=t0[:], scalar1=MAGIC,
                                    scalar2=-MAGIC, op0=OP.add, op1=OP.add)
            # attractor = 4*rhe - 2; 2*attractor = 8*rhe - 4
            nc.vector.tensor_scalar(out=t0[:], in0=t0[:], scalar1=8.0,
                                    scalar2=-4.0, op0=OP.mult, op1=OP.add)
            nc.vector.tensor_sub(target[:], t0[:], marg[:])
            # p_ori = 1/sum(exp(lg - max))
            for g in range(G):
                nc.scalar.activation(expv[:, g, :], lg[:, g, :], AF.Exp,
                                     bias=negmax[:, g:g + 1], scale=1.0,
                                     accum_out=sumexp[:, g:g + 1])
            nc.vector.reciprocal(p_ori[:], sumexp[:])
            nc.vector.memset(m[:], 0.0)
            nc.vector.memset(v[:], 0.0)

            # ---------- Adam loop ----------
            with (
                tc.tile_pool(name="loop", bufs=1) as lp,
                tc.tile_pool(name="lpsum", bufs=6, space=MemorySpace.PSUM) as lps,
            ):
                scr = lp.tile([P, G, C], f32)
                gs = lp.tile([P, G, C], f32)
                sq = lp.tile([P, G, C], f32)
                tA = lp.tile([P, G, C], f32r)
                tB = lp.tile([P, G, C], f32r)
                tU = lp.tile([P, G, C], f32r)
                tW = lp.tile([P, G, C], f32r)
                NT = ((0, 500), (500, 500))

                for t in range(1, num_iter + 1):
                    bc2 = 1.0 - BETA2 ** t
                    k4 = 1.0 / bc2

                    # stage-major emission: all groups per stage, so the
                    # scheduler's program-order priorities align with
                    # cross-group pipelining
                    for g in range(G):
                        gc = slice(g, g + 1)
                        nc.vector.max(out=v8[:, g, :], in_=lg[:, g, :])
                        nc.vector.tensor_scalar_mul(negmax[:, gc], v8[:, g, 0:1], -1.0)
                        nc.scalar.activation(expv[:, g, :], lg[:, g, :], AF.Exp,
                                             bias=negmax[:, gc], scale=1.0,
                                             accum_out=sumexp[:, gc])
                        # NB: tensor_tensor_reduce faults TRN2; this
                        # scalar_tensor_tensor+accum form is equivalent.
                        nc.vector.scalar_tensor_tensor(
                            out=scr[:, g, :], in0=expv[:, g, :], scalar=1.0,
                            in1=maskf[:, g, :], op0=OP.mult, op1=OP.mult,
                            accum_out=pn[:, gc],
                        )
                    for g in range(G):
                        gc = slice(g, g + 1)
                        nc.vector.reciprocal(rsum[:, gc], sumexp[:, gc])
                        nc.vector.tensor_mul(pstar[:, gc], pn[:, gc], rsum[:, gc])
                        nc.vector.tensor_sub(dcal[:, gc], pstar[:, gc], p_ori[:, gc])
                        nc.vector.tensor_sub(marg[:, gc], v8[:, g, 0:1], v8[:, g, 1:2])
                        nc.vector.tensor_sub(dmt[:, gc], marg[:, gc], target[:, gc])
                        nc.vector.tensor_scalar(out=sd[:, gc], in0=dmt[:, gc],
                                                scalar1=0.0, scalar2=2.0 * K1,
                                                op0=OP.is_ge, op1=OP.mult)
                        nc.vector.tensor_scalar_add(sd[:, gc], sd[:, gc], -K1)
                        nc.vector.tensor_scalar_mul(sdn[:, gc], sd[:, gc], -1.0)
                        nc.vector.tensor_scalar(out=t0[:, gc], in0=dcal[:, gc],
                                                scalar1=0.0, scalar2=2.0 * K2,
                                                op0=OP.is_ge, op1=OP.mult)
                        nc.vector.tensor_scalar_add(t0[:, gc], t0[:, gc], -K2)
                        nc.vector.tensor_mul(cc[:, gc], t0[:, gc], pstar[:, gc])
                        nc.vector.tensor_mul(t0[:, gc], cc[:, gc], rsum[:, gc])
                        nc.vector.tensor_scalar_mul(negr[:, gc], t0[:, gc], -1.0)
                    for g in range(G):
                        gc = slice(g, g + 1)
                        nc.vector.tensor_scalar(
                            out=tA[:, g, :], in0=lg[:, g, :],
                            scalar1=v8[:, g, 0:1], scalar2=sd[:, gc],
                            op0=OP.is_equal, op1=OP.mult,
                        )
                        nc.vector.tensor_scalar(
                            out=tB[:, g, :], in0=lg[:, g, :],
                            scalar1=v8[:, g, 1:2], scalar2=sdn[:, gc],
                            op0=OP.is_equal, op1=OP.mult,
                        )
                        nc.scalar.activation(tU[:, g, :], expv[:, g, :],
                                             AF.Copy, bias=0.0, scale=negr[:, gc])
                        nc.scalar.activation(tW[:, g, :], maskf[:, g, :],
                                             AF.Copy, bias=0.0, scale=cc[:, gc])
                    for g in range(G):
                        gc = slice(g, g + 1)
                        for n0, nw in NT:
                            gs_ps = lps.tile([P, 512], f32, tag="gs")
                            ns = slice(n0, n0 + nw)
                            nc.tensor.matmul(gs_ps[:, :nw], ident[:], tA[:, g, ns],
                                             start=True, stop=False)
                            nc.tensor.matmul(gs_ps[:, :nw], ident[:], tB[:, g, ns],
                                             start=False, stop=False)
                            nc.tensor.matmul(gs_ps[:, :nw], ident[:], tU[:, g, ns],
                                             start=False, stop=False)
                            nc.tensor.matmul(gs_ps[:, :nw], ident[:], tW[:, g, ns],
                                             start=False, stop=True)
                            nc.vector.scalar_tensor_tensor(
                                out=m[:, g, ns], in0=m[:, g, ns], scalar=BETA1,
                                in1=gs_ps[:, :nw], op0=OP.mult, op1=OP.add,
                            )
                            nc.scalar.activation(sq[:, g, ns], gs_ps[:, :nw],
                                                 AF.Square, bias=0.0, scale=K3)
                        nc.vector.scalar_tensor_tensor(
                            out=v[:, g, :], in0=v[:, g, :], scalar=BETA2,
                            in1=sq[:, g, :], op0=OP.mult, op1=OP.add,
                        )
                    for h in range(2):
                        hs = slice(2 * h, 2 * h + 2)
                        nc.scalar.activation(sq[:, hs, :], v[:, hs, :], AF.Ln,
                                             bias=0.0, scale=k4)
                        nc.scalar.activation(gs[:, hs, :], sq[:, hs, :], AF.Exp,
                                             bias=0.0, scale=0.5)
                        nc.scalar.activation(sq[:, hs, :], gs[:, hs, :], AF.Ln,
                                             bias=eps_sb[:], scale=1.0)
                        nc.scalar.activation(sq[:, hs, :], sq[:, hs, :], AF.Exp,
                                             bias=lnk5_sb[:, t - 1:t], scale=-1.0)
                        nc.vector.tensor_mul(gs[:, hs, :], m[:, hs, :], sq[:, hs, :])
                        nc.vector.tensor_sub(lg[:, hs, :], lg[:, hs, :], gs[:, hs, :])

            nc.sync.dma_start(out[:].rearrange("(g p) c -> p g c", p=P), lg[:])

    nc.compile()
    return nc


def _get_nc(num_iter=NUM_ITER):
    if num_iter not in _cache:
        _cache[num_iter] = build(num_iter)
    return _cache[num_iter]


def kernel(x, W, b, num_iter=NUM_ITER, trace=False):
    x = np.ascontiguousarray(x, dtype=np.float32)
    W = np.ascontiguousarray(W, dtype=np.float32)
    b = np.ascontiguousarray(b, dtype=np.float32).reshape(1, C)
    nc = _get_nc(num_iter)
    in_maps = []
    for i in range(NCORES):
        xt = np.ascontiguousarray(x[i * R:(i + 1) * R].T)
        in_maps.append({"xT": xt, "W": W, "b": b})
    res = run_bass_kernel_spmd(nc, in_maps, core_ids=list(range(NCORES)),
                               trace=trace)
    out = np.concatenate([res.results[i]["out"] for i in range(NCORES)], axis=0)
    if trace:
        kernel.last_exec_time_ns = res.exec_time_ns
        kernel.last_result = res
    return out


# revision 19
# speedup vs baseline: 1.2018x; 1.0234x over previous
"""Trainium2 Bass kernel: topk_masking Adam-loop (nn_AAALinear_6347961663813).

Data-parallel over 8 NeuronCores: each core takes 512 rows of x, computes
logits = x @ W + b (TensorE, float32r full-rate), then runs 100 fully
unrolled Adam iterations of the analytic gradient of
    loss = |margin(lg) - target|.mean() + 5*|softmax(lg)[j*] - p_ori|.mean()
entirely in SBUF. Layout: row r of the shard lives at partition r%128,
row-group g = r//128 (tiles are [128, 4, 1000]).

Analytic gradient per row (validated vs jax.grad):
    g = sd*(e_a - e_b) + cc*(e_j* - prob),  sd = sign+(margin-target)/B,
    cc = 5*sign+(pstar-p_ori)*pstar/B,      sign+(x) = x>=0 ? 1 : -1
where a/b are the current top-2 positions (realized as equality masks
against the top-2 values from the Max8 instruction) and j* is the fixed
original argmax. sqrt/reciprocal in the Adam denominator are computed as
exp/ln chains so the ScalarE table set never switches.
"""
import sys

for _p in ("/opt/trn_rl_repo", "/root/.axon_site/_ro/trn_rl_repo"):
    if _p not in sys.path:
        sys.path.insert(0, _p)

import numpy as np

import concourse.bass as bass
import concourse.mybir as mybir
import concourse.tile as tile
from concourse import bacc
from concourse.bass import MemorySpace
from concourse.bass_utils import run_bass_kernel_spmd

# All activation funcs we use (Exp, Ln, Square, Copy, Identity, Sign) live in
# the natural_log_exp_and_others table set. The default chooser alternates
# between exp_and_others and natural_log_exp_and_others, inserting ~6 table
# loads (~2.7us each) per loop iteration. Empty every other set's function
# list (keeping dict order, which defines act_func_set_id) so the chooser can
# only pick the combined set; the load then hoists out of the loop.
_orig_get_activation_tables = bacc.get_activation_tables


def _pinned_get_activation_tables(module_arch):
    tables = _orig_get_activation_tables(module_arch)
    pinned = {}
    for name, fns in tables.items():
        pinned[name] = fns if name == "natural_log_exp_and_others" else set()
    return pinned


bacc.get_activation_tables = _pinned_get_activation_tables

B, D, C = 4096, 2048, 1000
NCORES = 8
R = B // NCORES  # 512 rows per core
P = 128
G = R // P  # 4 row groups
KT = D // P  # 16 k-tiles
NUM_ITER = 100
LR, BETA1, BETA2, EPS, CAL_W = 0.1, 0.9, 0.999, 1e-8, 5.0

f32 = mybir.dt.float32
f32r = mybir.dt.float32r
AF = mybir.ActivationFunctionType
OP = mybir.AluOpType

_cache = {}


def build(num_iter=NUM_ITER):
    nc = bacc.Bacc(None, target_bir_lowering=False, debug=False)

    xT = nc.declare_dram_parameter("xT", [D, R], f32, isOutput=False)
    Wp = nc.declare_dram_parameter("W", [D, C], f32, isOutput=False)
    bp = nc.declare_dram_parameter("b", [1, C], f32, isOutput=False)
    out = nc.declare_dram_parameter("out", [R, C], f32, isOutput=True)

    K1 = (1.0 - BETA1) / B          # folds (1-beta1) of the m update into g
    K2 = CAL_W * (1.0 - BETA1) / B
    K3 = float(np.sqrt(1.0 - BETA2) / (1.0 - BETA1))  # Square(K3*gs) = (1-b2)*g^2

    with tile.TileContext(nc) as tc:
        with tc.tile_pool(name="persist", bufs=1) as pp:
            lg = pp.tile([P, G, C], f32)
            m = pp.tile([P, G, C], f32)
            v = pp.tile([P, G, C], f32)
            maskf = pp.tile([P, G, C], f32)
            expv = pp.tile([P, G, C], f32)
            v8 = pp.tile([P, G, 8], f32)
            # row scalars [P, G]
            negmax = pp.tile([P, G], f32)
            sumexp = pp.tile([P, G], f32)
            p_ori = pp.tile([P, G], f32)
            target = pp.tile([P, G], f32)
            pn = pp.tile([P, G], f32)
            rsum = pp.tile([P, G], f32)
            pstar = pp.tile([P, G], f32)
            dcal = pp.tile([P, G], f32)
            marg = pp.tile([P, G], f32)
            dmt = pp.tile([P, G], f32)
            sd = pp.tile([P, G], f32)
            sdn = pp.tile([P, G], f32)
            cc = pp.tile([P, G], f32)
            negr = pp.tile([P, G], f32)
            t0 = pp.tile([P, G], f32)
            # per-iteration bias constants ln(LR / (1 - beta1^t))
            lnk5_sb = pp.tile([P, num_iter], f32)
            for t in range(1, num_iter + 1):
                val = float(np.log(LR / (1.0 - BETA1 ** t)))
                nc.vector.memset(lnk5_sb[:, t - 1:t], val)
            eps_sb = pp.tile([P, 1], f32)
            nc.vector.memset(eps_sb[:], EPS)
            ident_f = pp.tile([P, P], f32)
            ident = pp.tile([P, P], f32r)
            from concourse.masks import make_identity
            make_identity(nc, ident_f[:])
            nc.vector.tensor_copy(ident[:], ident_f[:])

            # ---------- matmul: logits = x @ W + b ----------
            with (
                tc.tile_pool(name="mm", bufs=1) as mmp,
                tc.tile_pool(name="psum", bufs=4, space=MemorySpace.PSUM) as psp,
            ):
                Wr = Wp[:].rearrange("(k p) c -> p k c", p=P)
                Xr = xT[:].rearrange("(k p) r -> p k r", p=P)
                wr_sb = mmp.tile([P, KT, C], f32r)
                xr_sb = mmp.tile([P, KT, R], f32r)
                b_sb = mmp.tile([1, C], f32)
                nc.sync.dma_start(b_sb[:], bp[:])
                ones = mmp.tile([1, P], f32)
                nc.vector.memset(ones[:], 1.0)
                br_sb = mmp.tile([1, C], f32r)
                oner = mmp.tile([1, P], f32r)
                nc.vector.tensor_copy(br_sb[:], b_sb[:])
                nc.vector.tensor_copy(oner[:], ones[:])
                for k in range(KT):
                    sw = mmp.tile([P, C], f32, tag="sw", bufs=3)
                    nc.sync.dma_start(sw[:], Wr[:, k, :])
                    nc.vector.tensor_copy(wr_sb[:, k, :], sw[:])
                    sx = mmp.tile([P, R], f32, tag="sx", bufs=3)
                    nc.sync.dma_start(sx[:], Xr[:, k, :])
                    nc.vector.tensor_copy(xr_sb[:, k, :], sx[:])
                for g in range(G):
                    for n0, nw in ((0, 500), (500, 500)):
                        ps = psp.tile([P, 512], f32, tag="ps")
                        for k in range(KT):
                            nc.tensor.matmul(
                                ps[:, :nw],
                                xr_sb[:, k, g * P:(g + 1) * P],
                                wr_sb[:, k, n0:n0 + nw],
                                start=(k == 0),
                                stop=False,
                            )
                        nc.tensor.matmul(
                            ps[:, :nw],
                            oner[:],
                            br_sb[:, n0:n0 + nw],
                            start=False,
                            stop=True,
                        )
                        nc.scalar.copy(lg[:, g, n0:n0 + nw], ps[:, :nw])

            # ---------- setup: maskf, target, p_ori ----------
            for g in range(G):
                nc.vector.max(out=v8[:, g, :], in_=lg[:, g, :])
            nc.vector.tensor_scalar_mul(negmax[:], v8[:, :, 0], -1.0)
            for g in range(G):
                nc.vector.tensor_scalar(
                    out=maskf[:, g, :], in0=lg[:, g, :],
                    scalar1=v8[:, g, 0:1], scalar2=None, op0=OP.is_equal,
                )
            # target = 2*attractor - margin; attractor = 4*round_half_even(q) - 2,
            # q = margin/4 + 0.5. round_half_even via the fp32 (x+2^23)-2^23 trick.
            MAGIC = float(2 ** 23)
            nc.vector.tensor_sub(marg[:], v8[:, :, 0], v8[:, :, 1])
            nc.vector.tensor_scalar(out=t0[:], in0=marg[:], scalar1=0.25,
                                    scalar2=0.5, op0=OP.mult, op1=OP.add)
            nc.vector.tensor_scalar(out=t0[:], in0=t0[:], scalar1=MAGIC,
                                    scalar2=-MAGIC, op0=OP.add, op1=OP.add)
            # attractor = 4*rhe - 2; 2*attractor = 8*rhe - 4
            nc.vector.tensor_scalar(out=t0[:], in0=t0[:], scalar1=8.0,
                                    scalar2=-4.0, op0=OP.mult, op1=OP.add)
            nc.vector.tensor_sub(target[:], t0[:], marg[:])
            # p_ori = 1/sum(exp(lg - max))
            for g in range(G):
                nc.scalar.activation(expv[:, g, :], lg[:, g, :], AF.Exp,
                                     bias=negmax[:, g:g + 1], scale=1.0,
                                     accum_out=sumexp[:, g:g + 1])
            nc.vector.reciprocal(p_ori[:], sumexp[:])
            nc.vector.memset(m[:], 0.0)
            nc.vector.memset(v[:], 0.0)

            # ---------- Adam loop ----------
            with (
                tc.tile_pool(name="loop", bufs=1) as lp,
                tc.tile_pool(name="lpsum", bufs=6, space=MemorySpace.PSUM) as lps,
            ):
                scr = lp.tile([P, G, C], f32)
                gs = lp.tile([P, G, C], f32)
                sq = lp.tile([P, G, C], f32)
                tA = lp.tile([P, G, C], f32r)
                tB = lp.tile([P, G, C], f32r)
                tU = lp.tile([P, G, C], f32r)
                tW = lp.tile([P, G, C], f32r)
                NT = ((0, 500), (500, 500))

                for t in range(1, num_iter + 1):
                    bc2 = 1.0 - BETA2 ** t
                    k4 = 1.0 / bc2

                    # stage-major emission: all groups per stage, so the
                    # scheduler's program-order priorities align with
                    # cross-group pipelining
                    for g in range(G):
                        gc = slice(g, g + 1)
                        nc.vector.max(out=v8[:, g, :], in_=lg[:, g, :])
                        nc.vector.tensor_scalar_mul(negmax[:, gc], v8[:, g, 0:1], -1.0)
                        nc.scalar.activation(expv[:, g, :], lg[:, g, :], AF.Exp,
                                             bias=negmax[:, gc], scale=1.0,
                                             accum_out=sumexp[:, gc])
                        # NB: tensor_tensor_reduce faults TRN2; this
                        # scalar_tensor_tensor+accum form is equivalent.
                        nc.vector.scalar_tensor_tensor(
                            out=scr[:, g, :], in0=expv[:, g, :], scalar=1.0,
                            in1=maskf[:, g, :], op0=OP.mult, op1=OP.mult,
                            accum_out=pn[:, gc],
                        )
                    for g in range(G):
                        gc = slice(g, g + 1)
                        nc.vector.reciprocal(rsum[:, gc], sumexp[:, gc])
                        nc.vector.tensor_mul(pstar[:, gc], pn[:, gc], rsum[:, gc])
                        nc.vector.tensor_sub(dcal[:, gc], pstar[:, gc], p_ori[:, gc])
                        nc.vector.tensor_sub(marg[:, gc], v8[:, g, 0:1], v8[:, g, 1:2])
                        nc.vector.tensor_sub(dmt[:, gc], marg[:, gc], target[:, gc])
                        nc.vector.tensor_scalar(out=sd[:, gc], in0=dmt[:, gc],
                                                scalar1=0.0, scalar2=2.0 * K1,
                                                op0=OP.is_ge, op1=OP.mult)
                        nc.vector.tensor_scalar_add(sd[:, gc], sd[:, gc], -K1)
                        nc.vector.tensor_scalar_mul(sdn[:, gc], sd[:, gc], -1.0)
                        nc.vector.tensor_scalar(out=t0[:, gc], in0=dcal[:, gc],
                                                scalar1=0.0, scalar2=2.0 * K2,
                                                op0=OP.is_ge, op1=OP.mult)
                        nc.vector.tensor_scalar_add(t0[:, gc], t0[:, gc], -K2)
                        nc.vector.tensor_mul(cc[:, gc], t0[:, gc], pstar[:, gc])
                        nc.vector.tensor_mul(t0[:, gc], cc[:, gc], rsum[:, gc])
                        nc.vector.tensor_scalar_mul(negr[:, gc], t0[:, gc], -1.0)
                    for g in range(G):
                        gc = slice(g, g + 1)
                        nc.vector.tensor_scalar(
                            out=tA[:, g, :], in0=lg[:, g, :],
                            scalar1=v8[:, g, 0:1], scalar2=sd[:, gc],
                            op0=OP.is_equal, op1=OP.mult,
                        )
                        nc.vector.tensor_scalar(
                            out=tB[:, g, :], in0=lg[:, g, :],
                            scalar1=v8[:, g, 1:2], scalar2=sdn[:, gc],
                            op0=OP.is_equal, op1=OP.mult,
                        )
                        nc.scalar.activation(tU[:, g, :], expv[:, g, :],
                                             AF.Copy, bias=0.0, scale=negr[:, gc])
                        nc.scalar.activation(tW[:, g, :], maskf[:, g, :],
                                             AF.Copy, bias=0.0, scale=cc[:, gc])
                    for g in range(G):
                        gc = slice(g, g + 1)
                        for n0, nw in NT:
                            gs_ps = lps.tile([P, 512], f32, tag="gs")
                            ns = slice(n0, n0 + nw)
                            nc.tensor.matmul(gs_ps[:, :nw], ident[:], tA[:, g, ns],
                                             start=True, stop=False)
                            nc.tensor.matmul(gs_ps[:, :nw], ident[:], tB[:, g, ns],
                                             start=False, stop=False)
                            nc.tensor.matmul(gs_ps[:, :nw], ident[:], tU[:, g, ns],
                                             start=False, stop=False)
                            nc.tensor.matmul(gs_ps[:, :nw], ident[:], tW[:, g, ns],
                                             start=False, stop=True)
                            nc.vector.scalar_tensor_tensor(
                                out=m[:, g, ns], in0=m[:, g, ns], scalar=BETA1,
                                in1=gs_ps[:, :nw], op0=OP.mult, op1=OP.add,
                            )
                            nc.scalar.activation(sq[:, g, ns], gs_ps[:, :nw],
                                                 AF.Square, bias=0.0, scale=K3)
                        nc.vector.scalar_tensor_tensor(
                            out=v[:, g, :], in0=v[:, g, :], scalar=BETA2,
                            in1=sq[:, g, :], op0=OP.mult, op1=OP.add,
                        )
                    for g in range(G):
                        nc.scalar.activation(sq[:, g, :], v[:, g, :], AF.Ln,
                                             bias=0.0, scale=k4)
                        nc.scalar.activation(gs[:, g, :], sq[:, g, :], AF.Exp,
                                             bias=0.0, scale=0.5)
                        nc.scalar.activation(sq[:, g, :], gs[:, g, :], AF.Ln,
                                             bias=eps_sb[:], scale=1.0)
                        nc.scalar.activation(sq[:, g, :], sq[:, g, :], AF.Exp,
                                             bias=lnk5_sb[:, t - 1:t], scale=-1.0)
                        nc.vector.tensor_mul(gs[:, g, :], m[:, g, :], sq[:, g, :])
                        nc.vector.tensor_sub(lg[:, g, :], lg[:, g, :], gs[:, g, :])

            nc.sync.dma_start(out[:].rearrange("(g p) c -> p g c", p=P), lg[:])

    nc.compile()
    return nc


def _get_nc(num_iter=NUM_ITER):
    if num_iter not in _cache:
        _cache[num_iter] = build(num_iter)
    return _cache[num_iter]


def kernel(x, W, b, num_iter=NUM_ITER, trace=False):
    x = np.ascontiguousarray(x, dtype=np.float32)
    W = np.ascontiguousarray(W, dtype=np.float32)
    b = np.ascontiguousarray(b, dtype=np.float32).reshape(1, C)
    nc = _get_nc(num_iter)
    in_maps = []
    for i in range(NCORES):
        xt = np.ascontiguousarray(x[i * R:(i + 1) * R].T)
        in_maps.append({"xT": xt, "W": W, "b": b})
    res = run_bass_kernel_spmd(nc, in_maps, core_ids=list(range(NCORES)),
                               trace=trace)
    out = np.concatenate([res.results[i]["out"] for i in range(NCORES)], axis=0)
    if trace:
        kernel.last_exec_time_ns = res.exec_time_ns
        kernel.last_result = res
    return out
